# revision 11
# baseline (speedup 1.0000x reference)
"""Trainium2 Bass kernel for nn_KattentionV4 (windowed K-attention).

Reference computation (per batch b):
  win[l, g, i]   = X[b, l+g, i]                (sliding windows, Lw = L-K+1)
  Kmat           = win.reshape(Lw, K*C)
  Q_W[l, h, d]   = einsum('lgi,ghoi->lhgo', win, Wr).reshape(Lw, H, K*C)
  logits[h,q,k]  = Q_W[q,h,:] . Kmat[k,:]
  probs          = softmax(logits, axis=-1)

Sharding: 8 cores = 2 (batch) x 4 (head groups of 8 heads).
Each core computes its (b, 8-head) slice of logits/probs/Q_W on device.
All device matmuls contract over D = K*C = 40:
  QWT_h  (40,Lw)   = BDW_h.T @ KmatT          (BDW_h block-diag of Wr[:,h])
  QW     (128,320) = KmatT[:,tile].T @ [BDW_0|...|BDW_7]   (fp32, exact)
  logits (128,Lw)  = QWT_h[:,tile].T @ KmatT               (float32r, 4x PE rate)
f32r needs an even moving dim, so KmatT is host-padded to 1016 columns.
Softmax is fused: ACT exp(PSUM)->SBUF with accum_out row sums, DVE
reciprocal + scale; the raw-logit PSUM->SBUF copies alternate DVE/ACT.
Stores are grouped 4 heads at a time into ~2MB DMAs via a permuted DRAM
access pattern; logits stores issue from Sync (HWDGE), probs stores from
GpSimd (SWDGE) to spread queue/issue load. QWT prep is interleaved with
the store loop so DMA starts immediately. Kmat output is a pure
host-side window view of X.
"""

import os
import sys

if "/opt/trn_rl_repo" not in sys.path:
    sys.path.insert(0, "/opt/trn_rl_repo")

import numpy as np

B, L, C, K, H = 2, 1024, 4, 10, 32
LW = L - K + 1          # 1015
D = K * C               # 40
HPC = 8                 # heads per core
GH = 4                  # heads per store group
NCORES = 8
QT = 128                # q-tile partition rows
NT = (LW + QT - 1) // QT  # 8 q-tiles (last has 119 rows)

_cache = {}


def _build():
    import concourse.bacc as bacc
    import concourse.mybir as mybir
    from concourse import tile

    f32 = mybir.dt.float32
    f32r = mybir.dt.float32r
    Exp = mybir.ActivationFunctionType.Exp

    nc = bacc.Bacc(None, target_bir_lowering=False)
    kmt_d = nc.declare_dram_parameter("kmt", [D, LW + 1], f32, isOutput=False)
    bdw_d = nc.declare_dram_parameter("bdw", [D, HPC * D], f32, isOutput=False)
    logits_d = nc.declare_dram_parameter("logits", [HPC, LW, LW], f32, isOutput=True)
    probs_d = nc.declare_dram_parameter("probs", [HPC, LW, LW], f32, isOutput=True)
    qw_d = nc.declare_dram_parameter("qw", [LW, HPC * D], f32, isOutput=True)

    with tile.TileContext(nc) as tc:
        with (
            tc.tile_pool(name="const", bufs=1) as cpool,
            tc.tile_pool(name="ps", bufs=3, space="PSUM") as qps,
            tc.tile_pool(name="qwps", bufs=2, space="PSUM") as qwps,
            tc.tile_pool(name="sb", bufs=4) as spool,
            tc.tile_pool(name="small", bufs=12) as smpool,
        ):
            kmt = cpool.tile([D, LW + 1], f32)
            bdw = cpool.tile([D, HPC * D], f32)
            kmt_r = cpool.tile([D, LW + 1], f32r)
            bdw_r = cpool.tile([D, HPC * D], f32r)
            qwt_r = cpool.tile([D, HPC * LW], f32r)
            nc.sync.dma_start(kmt[:], kmt_d[:])
            nc.sync.dma_start(bdw[:], bdw_d[:])
            nc.vector.tensor_copy(kmt_r[:], kmt[:])
            nc.vector.tensor_copy(bdw_r[:], bdw[:])

            def emit_qw_tile(t):
                # Q_W output staging in (l, h, d) layout (fp32, exact); these
                # tiny stores are slotted mid-kernel to fill DMA gaps.
                t0 = t * QT
                sz = min(QT, LW - t0)
                ps = qwps.tile([QT, HPC * D], f32, tag="qw_ps")
                nc.tensor.matmul(ps[:sz], kmt[:, t0 : t0 + sz], bdw[:, 0 : HPC * D])
                sb = smpool.tile([QT, HPC * D], f32, tag="qw_sb")
                nc.vector.tensor_copy(sb[:sz], ps[:sz])
                nc.sync.dma_start(qw_d[t0 : t0 + sz, :], sb[:sz])

            def emit_qwt(h):
                # QWT_h = BDW_h.T @ KmatT (f32r)
                ps = qps.tile([D, 1024], f32, tag="lg_ps")
                lhsT = bdw_r[:, h * D : (h + 1) * D]
                nc.tensor.matmul(ps[:, 0:512], lhsT, kmt_r[:, 0:512])
                nc.tensor.matmul(ps[:, 512:1016], lhsT, kmt_r[:, 512 : LW + 1])
                nc.vector.tensor_copy(qwt_r[:, h * LW : (h + 1) * LW], ps[:, 0:LW])

            for hg in range(HPC // GH):
                for j in range(GH):
                    emit_qwt(hg * GH + j)
                for t in range(NT):
                    t0 = t * QT
                    sz = min(QT, LW - t0)
                    lg_big = spool.tile([QT, GH, LW], f32, tag="lg")
                    pr_big = spool.tile([QT, GH, LW], f32, tag="pr")
                    sum_sb = smpool.tile([QT, GH], f32, tag="sum")
                    rec_sb = smpool.tile([QT, GH], f32, tag="rec")
                    for j in range(GH):
                        h = hg * GH + j
                        lg_ps = qps.tile([QT, 1024], f32, tag="lg_ps")
                        lhsT = qwt_r[:, h * LW + t0 : h * LW + t0 + sz]
                        nc.tensor.matmul(lg_ps[:sz, 0:512], lhsT, kmt_r[:, 0:512])
                        nc.tensor.matmul(
                            lg_ps[:sz, 512:1016], lhsT, kmt_r[:, 512 : LW + 1]
                        )

                        # Raw-logits copy PSUM->SBUF: alternate DVE/ACT to
                        # balance engine load.
                        if (t + j) % 2 == 0:
                            nc.vector.tensor_copy(lg_big[:sz, j, :], lg_ps[:sz, 0:LW])
                        else:
                            nc.scalar.copy(lg_big[:sz, j, :], lg_ps[:sz, 0:LW])
                        nc.scalar.activation(
                            pr_big[:sz, j, :],
                            lg_ps[:sz, 0:LW],
                            Exp,
                            accum_out=sum_sb[:sz, j : j + 1],
                        )
                    nc.vector.reciprocal(rec_sb[:sz], sum_sb[:sz])
                    for j in range(GH):
                        nc.vector.tensor_scalar_mul(
                            pr_big[:sz, j, :],
                            pr_big[:sz, j, :],
                            rec_sb[:sz, j : j + 1],
                        )
                    h0 = hg * GH
                    out_lg = logits_d[h0 : h0 + GH, t0 : t0 + sz, :].rearrange(
                        "h p k -> p h k"
                    )
                    nc.sync.dma_start(out_lg, lg_big[:sz])
                    out_pr = probs_d[h0 : h0 + GH, t0 : t0 + sz, :].rearrange(
                        "h p k -> p h k"
                    )
                    nc.gpsimd.dma_start(out_pr, pr_big[:sz])

            for t in range(NT):
                emit_qw_tile(t)

    nc.finalize()
    return nc


def _get_nc():
    if "nc" not in _cache:
        _cache["nc"] = _build()
    return _cache["nc"]


def kernel(X, W):
    from concourse.bass_utils import run_bass_kernel_spmd

    X = np.ascontiguousarray(np.asarray(X, dtype=np.float32))
    W = np.ascontiguousarray(np.asarray(W, dtype=np.float32))
    Wr = W.reshape(K, H, C, C)

    in_maps = []
    for r in range(NCORES):
        b, h0 = r // 4, (r % 4) * HPC
        kmt = np.zeros((D, LW + 1), np.float32)
        for g in range(K):
            kmt[g * C : (g + 1) * C, 0:LW] = X[b, g : g + LW, :].T
        bdw = np.zeros((D, HPC * D), np.float32)
        for j in range(HPC):
            for g in range(K):
                c0 = j * D + g * C
                bdw[g * C : (g + 1) * C, c0 : c0 + C] = Wr[g, h0 + j].T
        in_maps.append({"kmt": kmt, "bdw": bdw})

    trace = bool(int(os.environ.get("KERNEL_TRACE", "0")))
    if trace:
        import profshim

        profshim.install()
    res = run_bass_kernel_spmd(
        _get_nc(), in_maps, core_ids=list(range(NCORES)), trace=trace
    )
    _cache["last_result"] = res

    probs = np.empty((B, H, LW, LW), np.float32)
    logits = np.empty((B, H, LW, LW), np.float32)
    qw = np.empty((B, LW, H, D), np.float32)
    for r in range(NCORES):
        b, h0 = r // 4, (r % 4) * HPC
        out = res.results[r]
        logits[b, h0 : h0 + HPC] = out["logits"]
        probs[b, h0 : h0 + HPC] = out["probs"]
        qw[b, :, h0 : h0 + HPC, :] = out["qw"].reshape(LW, HPC, D)

    idx = np.arange(LW)[:, None] + np.arange(K)[None, :]
    kmat = np.empty((B, LW, D), np.float32)
    for b in range(B):
        kmat[b] = X[b][idx].reshape(LW, D)

    return probs, logits, qw, kmat


# revision 12
# speedup vs baseline: 1.2839x; 1.2839x over previous
"""Trainium2 Bass kernel for nn_KattentionV4 (windowed K-attention).

Reference computation (per batch b):
  win[l, g, i]   = X[b, l+g, i]                (sliding windows, Lw = L-K+1)
  Kmat           = win.reshape(Lw, K*C)
  Q_W[l, h, d]   = einsum('lgi,ghoi->lhgo', win, Wr).reshape(Lw, H, K*C)
  logits[h,q,k]  = Q_W[q,h,:] . Kmat[k,:]
  probs          = softmax(logits, axis=-1)

Sharding: 8 cores = 2 (batch) x 4 (head groups of 8 heads).
Each core computes its (b, 8-head) slice of logits/probs/Q_W on device.
All device matmuls contract over D = K*C = 40:
  QWT_h  (40,Lw)   = BDW_h.T @ KmatT          (BDW_h block-diag of Wr[:,h])
  QW     (128,320) = KmatT[:,tile].T @ [BDW_0|...|BDW_7]   (fp32, exact)
  logits (128,Lw)  = QWT_h[:,tile].T @ KmatT               (float32r, 4x PE rate)
f32r needs an even moving dim, so KmatT is host-padded to 1016 columns.
Softmax is fused: ACT exp(PSUM)->SBUF with accum_out row sums, DVE
reciprocal + scale; the raw-logit PSUM->SBUF copies alternate DVE/ACT.
Stores are grouped 4 heads at a time into ~2MB DMAs via a permuted DRAM
access pattern; logits stores issue from Sync (HWDGE), probs stores from
GpSimd (SWDGE) to spread queue/issue load. QWT prep is interleaved with
the store loop so DMA starts immediately. Kmat output is a pure
host-side window view of X.
"""

import os
import sys

if "/opt/trn_rl_repo" not in sys.path:
    sys.path.insert(0, "/opt/trn_rl_repo")

import numpy as np

B, L, C, K, H = 2, 1024, 4, 10, 32
LW = L - K + 1          # 1015
D = K * C               # 40
HPC = 8                 # heads per core
GH = 4                  # heads per store group
NCORES = 8
QT = 128                # q-tile partition rows
NT = (LW + QT - 1) // QT  # 8 q-tiles (last has 119 rows)

_cache = {}


def _build():
    import concourse.bacc as bacc
    import concourse.mybir as mybir
    from concourse import tile

    f32 = mybir.dt.float32
    f32r = mybir.dt.float32r
    Exp = mybir.ActivationFunctionType.Exp

    nc = bacc.Bacc(None, target_bir_lowering=False)
    kmt_d = nc.declare_dram_parameter("kmt", [D, LW + 1], f32, isOutput=False)
    bdw_d = nc.declare_dram_parameter("bdw", [D, HPC * D], f32, isOutput=False)
    logits_d = nc.declare_dram_parameter("logits", [HPC, LW, LW], f32, isOutput=True)
    probs_d = nc.declare_dram_parameter("probs", [HPC, LW, LW], f32, isOutput=True)
    qw_d = nc.declare_dram_parameter("qw", [LW, HPC * D], f32, isOutput=True)

    with tile.TileContext(nc) as tc:
        with (
            tc.tile_pool(name="const", bufs=1) as cpool,
            tc.tile_pool(name="ps", bufs=3, space="PSUM") as qps,
            tc.tile_pool(name="qwps", bufs=2, space="PSUM") as qwps,
            tc.tile_pool(name="sb", bufs=3) as spool,
            tc.tile_pool(name="small", bufs=12) as smpool,
        ):
            kmt = cpool.tile([D, LW + 1], f32)
            bdw = cpool.tile([D, HPC * D], f32)
            kmt_r = cpool.tile([D, LW + 1], f32r)
            bdw_r = cpool.tile([D, HPC * D], f32r)
            qwt_r = cpool.tile([D, HPC * LW], f32r)
            nc.sync.dma_start(kmt[:], kmt_d[:])
            nc.sync.dma_start(bdw[:], bdw_d[:])
            nc.vector.tensor_copy(kmt_r[:], kmt[:])
            nc.vector.tensor_copy(bdw_r[:], bdw[:])

            def emit_qw_tile(t):
                # Q_W output staging in (l, h, d) layout (fp32, exact); these
                # tiny stores are slotted mid-kernel to fill DMA gaps.
                t0 = t * QT
                sz = min(QT, LW - t0)
                ps = qwps.tile([QT, HPC * D], f32, tag="qw_ps")
                nc.tensor.matmul(ps[:sz], kmt[:, t0 : t0 + sz], bdw[:, 0 : HPC * D])
                sb = smpool.tile([QT, HPC * D], f32, tag="qw_sb")
                nc.vector.tensor_copy(sb[:sz], ps[:sz])
                nc.sync.dma_start(qw_d[t0 : t0 + sz, :], sb[:sz])

            def emit_qwt(h):
                # QWT_h = BDW_h.T @ KmatT (f32r)
                ps = qps.tile([D, 1024], f32, tag="lg_ps")
                lhsT = bdw_r[:, h * D : (h + 1) * D]
                nc.tensor.matmul(ps[:, 0:512], lhsT, kmt_r[:, 0:512])
                nc.tensor.matmul(ps[:, 512:1016], lhsT, kmt_r[:, 512 : LW + 1])
                nc.vector.tensor_copy(qwt_r[:, h * LW : (h + 1) * LW], ps[:, 0:LW])

            for hg in range(HPC // GH):
                for j in range(GH):
                    emit_qwt(hg * GH + j)
                for t in range(NT):
                    t0 = t * QT
                    sz = min(QT, LW - t0)
                    lg_big = spool.tile([QT, GH, LW], f32, tag="lg")
                    pr_big = spool.tile([QT, GH, LW], f32, tag="pr")
                    sum_sb = smpool.tile([QT, GH], f32, tag="sum")
                    rec_sb = smpool.tile([QT, GH], f32, tag="rec")
                    for j in range(GH):
                        h = hg * GH + j
                        lg_ps = qps.tile([QT, 1024], f32, tag="lg_ps")
                        lhsT = qwt_r[:, h * LW + t0 : h * LW + t0 + sz]
                        nc.tensor.matmul(lg_ps[:sz, 0:512], lhsT, kmt_r[:, 0:512])
                        nc.tensor.matmul(
                            lg_ps[:sz, 512:1016], lhsT, kmt_r[:, 512 : LW + 1]
                        )

                        # Raw-logits copy PSUM->SBUF: alternate DVE/ACT to
                        # balance engine load.
                        if (t + j) % 2 == 0:
                            nc.vector.tensor_copy(lg_big[:sz, j, :], lg_ps[:sz, 0:LW])
                        else:
                            nc.scalar.copy(lg_big[:sz, j, :], lg_ps[:sz, 0:LW])
                        nc.scalar.activation(
                            pr_big[:sz, j, :],
                            lg_ps[:sz, 0:LW],
                            Exp,
                            accum_out=sum_sb[:sz, j : j + 1],
                        )
                    nc.vector.reciprocal(rec_sb[:sz], sum_sb[:sz])
                    for j in range(GH):
                        nc.vector.tensor_scalar_mul(
                            pr_big[:sz, j, :],
                            pr_big[:sz, j, :],
                            rec_sb[:sz, j : j + 1],
                        )
                    h0 = hg * GH
                    out_lg = logits_d[h0 : h0 + GH, t0 : t0 + sz, :].rearrange(
                        "h p k -> p h k"
                    )
                    nc.sync.dma_start(out_lg, lg_big[:sz])
                    out_pr = probs_d[h0 : h0 + GH, t0 : t0 + sz, :].rearrange(
                        "h p k -> p h k"
                    )
                    nc.gpsimd.dma_start(out_pr, pr_big[:sz])

            for t in range(NT):
                emit_qw_tile(t)

    nc.finalize()
    return nc


def _get_nc():
    if "nc" not in _cache:
        _cache["nc"] = _build()
    return _cache["nc"]


def kernel(X, W):
    from concourse.bass_utils import run_bass_kernel_spmd

    X = np.ascontiguousarray(np.asarray(X, dtype=np.float32))
    W = np.ascontiguousarray(np.asarray(W, dtype=np.float32))
    Wr = W.reshape(K, H, C, C)

    in_maps = []
    for r in range(NCORES):
        b, h0 = r // 4, (r % 4) * HPC
        kmt = np.zeros((D, LW + 1), np.float32)
        for g in range(K):
            kmt[g * C : (g + 1) * C, 0:LW] = X[b, g : g + LW, :].T
        bdw = np.zeros((D, HPC * D), np.float32)
        for j in range(HPC):
            for g in range(K):
                c0 = j * D + g * C
                bdw[g * C : (g + 1) * C, c0 : c0 + C] = Wr[g, h0 + j].T
        in_maps.append({"kmt": kmt, "bdw": bdw})

    trace = bool(int(os.environ.get("KERNEL_TRACE", "0")))
    if trace:
        import profshim

        profshim.install()
    res = run_bass_kernel_spmd(
        _get_nc(), in_maps, core_ids=list(range(NCORES)), trace=trace
    )
    _cache["last_result"] = res

    probs = np.empty((B, H, LW, LW), np.float32)
    logits = np.empty((B, H, LW, LW), np.float32)
    qw = np.empty((B, LW, H, D), np.float32)
    for r in range(NCORES):
        b, h0 = r // 4, (r % 4) * HPC
        out = res.results[r]
        logits[b, h0 : h0 + HPC] = out["logits"]
        probs[b, h0 : h0 + HPC] = out["probs"]
        qw[b, :, h0 : h0 + HPC, :] = out["qw"].reshape(LW, HPC, D)

    idx = np.arange(LW)[:, None] + np.arange(K)[None, :]
    kmat = np.empty((B, LW, D), np.float32)
    for b in range(B):
        kmat[b] = X[b][idx].reshape(LW, D)

    return probs, logits, qw, kmat
